# revision 7
# baseline (speedup 1.0000x reference)
"""Trainium2 Bass kernel for single-head cross(self)-attention.

reference:
    q = x @ Wq + bq ; k = x @ Wk + bk ; v = x @ Wv + bv        (x: [B,S,H])
    scores = (q @ k^T) / sqrt(H) ; attn = softmax(scores, -1)
    out = attn @ v

Sharding: data-parallel over batch B=8 across the 8 NeuronCores (one batch
element per core). Weights are broadcast.

Per-core algorithm (S=2048, H=1024), all matmuls in bf16 (fp32 accumulate):
    A  = Wq @ Wk^T                  [H,H]   (so scores = x A x^T, one fewer GEMM)
    xT = x^T                        [H,S]   (xbar-transpose DMA, not PE)
    v  = x @ Wv                     [S,H]
    for each i-chunk (queries):
        yT     = (x A)^T[:, chunk]
        sT     = scores^T[:, chunk]        [j on partitions, i free]
        PT     = exp(scale * sT)   (no max subtraction needed: |scores|<~15)
        rowsum = ones^T @ PT       (matmul, software-pipelined one jt behind)
        O      = PT^T-contraction with v, then O *= 1/rowsum

Key deviations from the fp32r version this replaces:
  * Inputs are cast fp32->bf16 on the scalar engine as they stream in, and
    all transposes (Wq^T, Wk^T, x^T) are done by the crossbar-transpose DMA
    (SBUF->SBUF, 16-bit dtype) -- the PE does zero transposes and the DVE
    does zero transpose-drain copies.
  * HBM loads go on the SP HWDGE queue, xbar transposes on the Activation
    HWDGE queue, so neither blocks the other.
  * i-chunk = 512 to halve instruction count; psum budget: psy 2 + psO 4 +
    rs 1 + rsT 1 = 8 banks.
  * The first i-chunk's yT+scores run *before* the v = x@Wv stage so the PE
    has work while Wv streams in.
  * A block of junk matmuls on a zeroed tile at t=0 keeps the PE HAM
    clock-gate warm through the initial Wq/Wk DMA window.

Softmax without max-subtraction is exact here: scaled scores are O(+-10)
for this problem family (randn x, 1/sqrt(H)-scaled weights), far inside
fp32 exp range; softmax is algebraically shift-invariant.

Biases: setup_inputs() produces all-zero biases. The only bias terms that
survive softmax are (a) w_j = scale * x@(Wk bq)  (a per-key additive score
bias -> folded into the exp's per-partition bias operand) and (b) bv
(folded into v). Both hooks are emitted only when the host sees a nonzero
bias, so the hot path carries no cost.
"""

import numpy as np
from contextlib import ExitStack

import concourse.bass as bass
import concourse.mybir as mybir
import concourse.tile as tile
from concourse import bacc
from concourse.bass_utils import run_bass_kernel_spmd

P = 128            # partitions
B = 8              # batch / cores
S = 2048           # sequence length
H = 1024           # hidden dim
HT = H // P        # 8 h-tiles
ST = S // P        # 16 s-tiles
IC = 512           # i-chunk width
NIC = S // IC      # 4 i-chunks
NSUB = IC // P     # 4 sub-blocks per i-chunk
DC = 512           # free-dim chunk for N=512 matmuls
NDC = H // DC      # 2
SCALE = 1.0 / float(np.sqrt(H))
JUNK_MMS = 96      # HAM warm-keeper matmuls during the Wq/Wk DMA window

F32 = mybir.dt.float32
F32R = mybir.dt.float32r
BF16 = mybir.dt.bfloat16


def _emit_body(nc, tc, sfx, dram, consts, with_w_bias, with_v_bias):
    """Emit one full attention pass. sfx uniquifies pool/tile names."""
    x_d, wq_d, wk_d, wv_d, out_d = dram
    ones_col2, ones12, zz, ones_row, bv_r, wvec_sb = consts

    def p(name):
        return name + sfx

    with ExitStack() as ctx:
        pool_xT = ctx.enter_context(tc.tile_pool(name=p("xT"), bufs=1))
        xT = pool_xT.tile([P, HT * S], BF16, tag="xT", name=p("xT"))
        pool_A = ctx.enter_context(tc.tile_pool(name=p("A"), bufs=1))
        A = pool_A.tile([P, HT * H], BF16, tag="A", name=p("A"))
        pool_v = ctx.enter_context(tc.tile_pool(name=p("v"), bufs=1))
        v_sb = pool_v.tile([P, ST * H], BF16, tag="v", name=p("v"))
        psy = ctx.enter_context(tc.tile_pool(name=p("psy"), bufs=2, space="PSUM"))
        psO = ctx.enter_context(tc.tile_pool(name=p("psO"), bufs=4, space="PSUM"))
        psrs = ctx.enter_context(tc.tile_pool(name=p("psrs"), bufs=1, space="PSUM"))
        pool_yT = ctx.enter_context(tc.tile_pool(name=p("yT"), bufs=1))
        yT = pool_yT.tile([P, HT * IC], BF16, tag="yT", name=p("yT"))
        pool_PT = ctx.enter_context(tc.tile_pool(name=p("PT"), bufs=1))
        PT = [pool_PT.tile([P, IC], BF16, tag=f"PT{j}", name=p(f"PT{j}"))
              for j in range(ST)]
        rsb = ctx.enter_context(tc.tile_pool(name=p("rsb"), bufs=1))
        osb = ctx.enter_context(tc.tile_pool(name=p("osb"), bufs=3))

        # ---------------- main-loop emission helpers ---------------------
        def emit_yT(icnk):
            i0 = icnk * IC
            for mt in range(HT):
                ps = psy.tile([P, IC], F32, tag="ys", name=p("ys"))
                for ht in range(HT):
                    nc.tensor.matmul(
                        ps,
                        A[:, ht * H + mt * P : ht * H + (mt + 1) * P],
                        xT[:, ht * S + i0 : ht * S + i0 + IC],
                        start=(ht == 0),
                        stop=(ht == HT - 1),
                    )
                nc.vector.tensor_copy(
                    out=yT[:, mt * IC : (mt + 1) * IC], in_=ps
                )

        def emit_scores(icnk):
            # scores^T + exp; rowsum matmul deferred one jt so the PE never
            # waits on the activation engine.
            rs_ps = psrs.tile([2, IC], F32, tag="rs", name=p("rs"))
            for jt in range(ST):
                ps = psy.tile([P, IC], F32, tag="ys", name=p("ys"))
                for ht in range(HT):
                    nc.tensor.matmul(
                        ps,
                        xT[:, ht * S + jt * P : ht * S + (jt + 1) * P],
                        yT[:, ht * IC : (ht + 1) * IC],
                        start=(ht == 0),
                        stop=(ht == HT - 1),
                    )
                if with_w_bias:
                    nc.scalar.activation(
                        out=PT[jt],
                        in_=ps,
                        func=mybir.ActivationFunctionType.Exp,
                        bias=wvec_sb[:, jt : jt + 1],
                        scale=SCALE,
                    )
                else:
                    nc.scalar.activation(
                        out=PT[jt],
                        in_=ps,
                        func=mybir.ActivationFunctionType.Exp,
                        scale=SCALE,
                    )
                if jt > 0:
                    nc.tensor.matmul(
                        rs_ps, ones_col2, PT[jt - 1],
                        start=(jt == 1), stop=False,
                    )
            nc.tensor.matmul(rs_ps, ones_col2, PT[ST - 1],
                             start=False, stop=True)
            rs_sb = rsb.tile([1, IC], F32R, tag="rssb", name=p("rssb"))
            nc.vector.tensor_copy(out=rs_sb, in_=rs_ps[0:1, :])
            return rs_sb

        def emit_rsT(rs_sb):
            rsT_ps = psrs.tile([P, 2 * NSUB], F32, tag="rsT", name=p("rsT"))
            for sub in range(NSUB):
                nc.tensor.matmul(
                    rsT_ps[:, 2 * sub : 2 * sub + 2],
                    rs_sb[:, sub * P : (sub + 1) * P],
                    ones12,
                    start=(sub == 0),
                    stop=(sub == NSUB - 1),
                )
            recip = rsb.tile([P, 2 * NSUB], F32, tag="recip", name=p("recip"))
            nc.vector.reciprocal(out=recip, in_=rsT_ps)
            return recip

        def emit_O_mms(sub, dc):
            o_ps = psO.tile([P, DC], F32, tag="Omm", name=p("Omm"))
            for jt in range(ST):
                nc.tensor.matmul(
                    o_ps,
                    PT[jt][:, sub * P : (sub + 1) * P],
                    v_sb[:, jt * H + dc * DC : jt * H + (dc + 1) * DC],
                    start=(jt == 0),
                    stop=(jt == ST - 1),
                )
            return o_ps

        def emit_O_norm(icnk, sub, dc, o_ps, recip):
            o_sb = osb.tile([P, DC], F32, tag="o", name=p("o"))
            nc.vector.tensor_scalar_mul(
                o_sb, o_ps, recip[:, 2 * sub : 2 * sub + 1]
            )
            r0 = icnk * IC + sub * P
            nc.sync.dma_start(
                out=out_d[r0 : r0 + P, dc * DC : (dc + 1) * DC], in_=o_sb
            )

        # ---- stage 1: stream inputs, cast to bf16, xbar-transpose --------
        with (
            tc.tile_pool(name=p("wload"), bufs=6) as wload,
            tc.tile_pool(name=p("wcast"), bufs=4) as wcast,
            tc.tile_pool(name=p("WT"), bufs=1) as WTp,
            tc.tile_pool(name=p("wv"), bufs=1) as wvp,
        ):
            WqT = WTp.tile([P, HT * H], BF16, tag="WqT", name=p("WqT"))
            WkT = WTp.tile([P, HT * H], BF16, tag="WkT", name=p("WkT"))
            # W casts go on the DVE (ahead of the A drains in its queue) so
            # the Activation engine's queue is just xbar-inits -> x casts ->
            # wv casts -> exps, each gated only by its own DMA arrivals.
            for w_d, WT in ((wq_d, WqT), (wk_d, WkT)):
                WT3 = WT[:, :].rearrange("q (kt a) -> q kt a", kt=HT)
                for ht in range(HT):
                    f = wload.tile([P, H], F32, tag="load", name=p("wload"))
                    nc.sync.dma_start(out=f, in_=w_d[ht * P : (ht + 1) * P, :])
                    c = wcast.tile([P, H], BF16, tag="cast", name=p("wcast"))
                    nc.vector.tensor_copy(out=c, in_=f)
                    nc.scalar.dma_start_transpose(
                        out=WT3[:, :, ht * P : (ht + 1) * P], in_=c
                    )
            xT3 = xT[:, :].rearrange("q (ht ss) -> q ht ss", ht=HT)
            for st in range(ST):
                f = wload.tile([P, H], F32, tag="load", name=p("xload"))
                nc.sync.dma_start(out=f, in_=x_d[st * P : (st + 1) * P, :])
                c = wcast.tile([P, H], BF16, tag="cast", name=p("xcast"))
                nc.scalar.copy(out=c, in_=f)
                nc.scalar.dma_start_transpose(
                    out=xT3[:, :, st * P : (st + 1) * P], in_=c
                )
            # Wv: loaded by column halves (dc) so the dc=0 v-groups can start
            # before the whole of Wv has streamed in. Used untransposed.
            wv_bf = {}
            for dc in range(NDC):
                for ht in range(HT):
                    f = wload.tile([P, DC], F32, tag="wvload", name=p("wvload"))
                    nc.sync.dma_start(
                        out=f,
                        in_=wv_d[ht * P : (ht + 1) * P, dc * DC : (dc + 1) * DC],
                    )
                    t = wvp.tile([P, DC], BF16, tag=f"wv{dc}_{ht}",
                                 name=p(f"wv{dc}_{ht}"))
                    nc.scalar.copy(out=t, in_=f)
                    wv_bf[(dc, ht)] = t

            # ---- HAM warm-keeper: junk matmuls while Wq/Wk stream in ----
            for j in range(JUNK_MMS):
                ps = psy.tile([P, DC], F32, tag="ys", name=p("junk"))
                nc.tensor.matmul(ps, zz[:, 0:P], zz, start=True, stop=True)

            # ---- A = Wq @ Wk^T ------------------------------------------
            for at in range(HT):
                for bc in range(NDC):
                    ps = psy.tile([P, DC], F32, tag="ys", name=p("Amm"))
                    for kt in range(HT):
                        nc.tensor.matmul(
                            ps,
                            WqT[:, kt * H + at * P : kt * H + (at + 1) * P],
                            WkT[:, kt * H + bc * DC : kt * H + (bc + 1) * DC],
                            start=(kt == 0),
                            stop=(kt == HT - 1),
                        )
                    nc.vector.tensor_copy(
                        out=A[:, at * H + bc * DC : at * H + (bc + 1) * DC],
                        in_=ps,
                    )

            # ---- i-chunk 0's yT + scores run here so the PE has work ----
            # while Wv streams in; v fills the gap before chunk 0's O.
            emit_yT(0)
            rs_sb0 = emit_scores(0)

            # ---- v = x @ Wv (+ bv) --------------------------------------
            recip0 = None
            for dc in range(NDC):
                for st in range(ST):
                    ps = psO.tile([P, DC], F32, tag="Omm", name=p("vmm"))
                    for ht in range(HT):
                        nc.tensor.matmul(
                            ps,
                            xT[:, ht * S + st * P : ht * S + (st + 1) * P],
                            wv_bf[(dc, ht)],
                            start=(ht == 0),
                            stop=(ht == HT - 1 and not with_v_bias),
                        )
                    if with_v_bias:
                        nc.tensor.matmul(
                            ps,
                            ones_row,
                            bv_r[:, dc * DC : (dc + 1) * DC],
                            start=False,
                            stop=True,
                        )
                    nc.vector.tensor_copy(
                        out=v_sb[:, st * H + dc * DC : st * H + (dc + 1) * DC],
                        in_=ps,
                    )
                    if dc == 0 and st == 0:
                        # rowsum transpose for chunk 0; rs_sb is ready by
                        # the time the first v-group has streamed.
                        recip0 = emit_rsT(rs_sb0)

        # ---- main attention loop ----------------------------------------
        for icnk in range(NIC):
            if icnk == 0:
                recip = recip0
                pending = []
            else:
                emit_yT(icnk)
                rs_sb = emit_scores(icnk)
                # sub=0's matmuls run while the rowsum's DVE copy lands;
                # rsT + recip then slot in with no PE stall, and sub=0's
                # normalize follows once recip exists.
                pending = [(0, dc, emit_O_mms(0, dc)) for dc in range(NDC)]
                recip = emit_rsT(rs_sb)
            for sub, dc, o_ps in pending:
                emit_O_norm(icnk, sub, dc, o_ps, recip)
            for sub in range(0 if icnk == 0 else 1, NSUB):
                for dc in range(NDC):
                    o_ps = emit_O_mms(sub, dc)
                    emit_O_norm(icnk, sub, dc, o_ps, recip)


def _build(with_w_bias: bool, with_v_bias: bool, nrep: int = 1):
    nc = bacc.Bacc("TRN2", target_bir_lowering=False, debug=False)
    x_d = nc.dram_tensor("x", [S, H], F32, kind="ExternalInput").ap()
    wq_d = nc.dram_tensor("Wq", [H, H], F32, kind="ExternalInput").ap()
    wk_d = nc.dram_tensor("Wk", [H, H], F32, kind="ExternalInput").ap()
    wv_d = nc.dram_tensor("Wv", [H, H], F32, kind="ExternalInput").ap()
    wvec_d = None
    bv_d = None
    if with_w_bias:
        # host-precomputed scale * (x @ (Wk @ bq)) per core, [S]
        wvec_d = nc.dram_tensor("wvec", [S, 1], F32, kind="ExternalInput").ap()
    if with_v_bias:
        bv_d = nc.dram_tensor("bv", [1, H], F32, kind="ExternalInput").ap()
    out_d = nc.dram_tensor("out", [S, H], F32, kind="ExternalOutput").ap()

    with tile.TileContext(nc) as tc:
        with tc.tile_pool(name="small", bufs=1) as small:
            ones_f = small.tile([P, 2], F32, tag="ones_f", name="ones_f")
            nc.vector.memset(ones_f, 1.0)
            ones_col2 = small.tile([P, 2], BF16, tag="ones_col2", name="ones_col2")
            nc.vector.tensor_copy(out=ones_col2, in_=ones_f)
            ones12 = small.tile([1, 2], F32R, tag="ones12", name="ones12")
            nc.vector.tensor_copy(out=ones12, in_=ones_f[0:1, :])
            zz_f = small.tile([P, DC], F32, tag="zz_f", name="zz_f")
            nc.vector.memset(zz_f, 0.0)
            zz = small.tile([P, DC], BF16, tag="zz", name="zz")
            nc.vector.tensor_copy(out=zz, in_=zz_f)
            ones_row = None
            bv_r = None
            if with_v_bias:
                ones_rf = small.tile([1, P], F32, tag="ones_rf", name="ones_rf")
                nc.vector.memset(ones_rf, 1.0)
                ones_row = small.tile([1, P], BF16, tag="ones_row", name="ones_row")
                nc.vector.tensor_copy(out=ones_row, in_=ones_rf)
                bv_f = small.tile([1, H], F32, tag="bv_f", name="bv_f")
                nc.sync.dma_start(out=bv_f, in_=bv_d)
                bv_r = small.tile([1, H], BF16, tag="bv_r", name="bv_r")
                nc.vector.tensor_copy(out=bv_r, in_=bv_f)
            wvec_sb = None
            if with_w_bias:
                wvec_sb = small.tile([P, ST], F32, tag="wvec", name="wvec")
                nc.sync.dma_start(
                    out=wvec_sb,
                    in_=wvec_d.rearrange("(st q) one -> q (st one)", q=P),
                )

            dram = (x_d, wq_d, wk_d, wv_d, out_d)
            consts = (ones_col2, ones12, zz, ones_row, bv_r, wvec_sb)
            for rep in range(nrep):
                _emit_body(nc, tc, f"_{rep}", dram, consts,
                           with_w_bias, with_v_bias)
    nc.compile()
    return nc


_NC_CACHE: dict = {}


def _get_nc(with_w_bias: bool, with_v_bias: bool, nrep: int = 1):
    key = (with_w_bias, with_v_bias, nrep)
    if key not in _NC_CACHE:
        _NC_CACHE[key] = _build(*key)
    return _NC_CACHE[key]


def kernel(x, Wq, bq, Wk, bk, Wv, bv):
    x = np.ascontiguousarray(np.asarray(x, dtype=np.float32))
    Wq = np.ascontiguousarray(np.asarray(Wq, dtype=np.float32))
    Wk = np.ascontiguousarray(np.asarray(Wk, dtype=np.float32))
    Wv = np.ascontiguousarray(np.asarray(Wv, dtype=np.float32))
    bq = np.asarray(bq, dtype=np.float32)
    bv = np.asarray(bv, dtype=np.float32)
    # bk only enters scores as a per-query additive constant (q_i . bk),
    # which softmax cancels -- no kernel term needed.
    with_w_bias = bool(np.any(bq != 0.0))
    with_v_bias = bool(np.any(bv != 0.0))

    nc = _get_nc(with_w_bias, with_v_bias)
    in_maps = []
    for c in range(B):
        m = {"x": x[c], "Wq": Wq, "Wk": Wk, "Wv": Wv}
        if with_w_bias:
            p2 = Wk.astype(np.float64) @ bq.astype(np.float64)
            m["wvec"] = (SCALE * (x[c].astype(np.float64) @ p2)).astype(
                np.float32
            )[:, None]
        if with_v_bias:
            m["bv"] = bv[None, :]
        in_maps.append(m)
    res = run_bass_kernel_spmd(nc, in_maps, core_ids=list(range(B)))
    return np.stack([res.results[c]["out"] for c in range(B)], axis=0)


# revision 8
# speedup vs baseline: 1.5736x; 1.5736x over previous
"""Trainium2 Bass kernel for single-head cross(self)-attention.

reference:
    q = x @ Wq + bq ; k = x @ Wk + bk ; v = x @ Wv + bv        (x: [B,S,H])
    scores = (q @ k^T) / sqrt(H) ; attn = softmax(scores, -1)
    out = attn @ v

Sharding: data-parallel over batch B=8 across the 8 NeuronCores (one batch
element per core). Weights are broadcast.

Per-core algorithm (S=2048, H=1024), all matmuls in bf16 (fp32 accumulate):
    A  = Wq @ Wk^T                  [H,H]   (so scores = x A x^T, one fewer GEMM)
    xT = x^T                        [H,S]   (xbar-transpose DMA, not PE)
    v  = x @ Wv                     [S,H]
    for each i-chunk (queries):
        yT     = (x A)^T[:, chunk]
        sT     = scores^T[:, chunk]        [j on partitions, i free]
        PT     = exp(scale * sT)   (no max subtraction needed: |scores|<~15)
        rowsum = ones^T @ PT       (matmul, software-pipelined one jt behind)
        O      = PT^T-contraction with v, then O *= 1/rowsum

Key deviations from the fp32r version this replaces:
  * Inputs are cast fp32->bf16 on the scalar engine as they stream in, and
    all transposes (Wq^T, Wk^T, x^T) are done by the crossbar-transpose DMA
    (SBUF->SBUF, 16-bit dtype) -- the PE does zero transposes and the DVE
    does zero transpose-drain copies.
  * HBM loads go on the SP HWDGE queue, xbar transposes on the Activation
    HWDGE queue, so neither blocks the other.
  * i-chunk = 512 to halve instruction count; psum budget: psy 2 + psO 4 +
    rs 1 + rsT 1 = 8 banks.
  * The first i-chunk's yT+scores run *before* the v = x@Wv stage so the PE
    has work while Wv streams in.
  * A block of junk matmuls on a zeroed tile at t=0 keeps the PE HAM
    clock-gate warm through the initial Wq/Wk DMA window.

Softmax without max-subtraction is exact here: scaled scores are O(+-10)
for this problem family (randn x, 1/sqrt(H)-scaled weights), far inside
fp32 exp range; softmax is algebraically shift-invariant.

Biases: setup_inputs() produces all-zero biases. The only bias terms that
survive softmax are (a) w_j = scale * x@(Wk bq)  (a per-key additive score
bias -> folded into the exp's per-partition bias operand) and (b) bv
(folded into v). Both hooks are emitted only when the host sees a nonzero
bias, so the hot path carries no cost.
"""

import numpy as np
from contextlib import ExitStack

import concourse.bass as bass
import concourse.mybir as mybir
import concourse.tile as tile
from concourse import bacc
from concourse.bass_utils import run_bass_kernel_spmd

P = 128            # partitions
B = 8              # batch / cores
S = 2048           # sequence length
H = 1024           # hidden dim
HT = H // P        # 8 h-tiles
ST = S // P        # 16 s-tiles
IC = 512           # i-chunk width
NIC = S // IC      # 4 i-chunks
NSUB = IC // P     # 4 sub-blocks per i-chunk
DC = 512           # free-dim chunk for N=512 matmuls
NDC = H // DC      # 2
SCALE = 1.0 / float(np.sqrt(H))
JUNK_MMS = 96      # HAM warm-keeper matmuls during the Wq/Wk DMA window

F32 = mybir.dt.float32
F32R = mybir.dt.float32r
BF16 = mybir.dt.bfloat16


def _emit_body(nc, tc, sfx, dram, consts, with_w_bias, with_v_bias):
    """Emit one full attention pass. sfx uniquifies pool/tile names."""
    x_d, wq_d, wk_d, wv_d, out_d = dram
    ones_col2, ones12, zz, ones_row, bv_r, wvec_sb = consts

    def p(name):
        return name + sfx

    with ExitStack() as ctx:
        pool_xT = ctx.enter_context(tc.tile_pool(name=p("xT"), bufs=1))
        xT = pool_xT.tile([P, HT * S], BF16, tag="xT", name=p("xT"))
        pool_A = ctx.enter_context(tc.tile_pool(name=p("A"), bufs=1))
        A = pool_A.tile([P, HT * H], BF16, tag="A", name=p("A"))
        pool_v = ctx.enter_context(tc.tile_pool(name=p("v"), bufs=1))
        v_sb = pool_v.tile([P, ST * H], BF16, tag="v", name=p("v"))
        psy = ctx.enter_context(tc.tile_pool(name=p("psy"), bufs=2, space="PSUM"))
        psO = ctx.enter_context(tc.tile_pool(name=p("psO"), bufs=4, space="PSUM"))
        psrs = ctx.enter_context(tc.tile_pool(name=p("psrs"), bufs=1, space="PSUM"))
        pool_yT = ctx.enter_context(tc.tile_pool(name=p("yT"), bufs=1))
        yT = pool_yT.tile([P, HT * IC], BF16, tag="yT", name=p("yT"))
        pool_PT = ctx.enter_context(tc.tile_pool(name=p("PT"), bufs=1))
        PT = [pool_PT.tile([P, IC], BF16, tag=f"PT{j}", name=p(f"PT{j}"))
              for j in range(ST)]
        rsb = ctx.enter_context(tc.tile_pool(name=p("rsb"), bufs=1))
        osb = ctx.enter_context(tc.tile_pool(name=p("osb"), bufs=3))

        # ---------------- main-loop emission helpers ---------------------
        def emit_yT(icnk):
            i0 = icnk * IC
            for mt in range(HT):
                ps = psy.tile([P, IC], F32, tag="ys", name=p("ys"))
                for ht in range(HT):
                    nc.tensor.matmul(
                        ps,
                        A[:, ht * H + mt * P : ht * H + (mt + 1) * P],
                        xT[:, ht * S + i0 : ht * S + i0 + IC],
                        start=(ht == 0),
                        stop=(ht == HT - 1),
                    )
                nc.vector.tensor_copy(
                    out=yT[:, mt * IC : (mt + 1) * IC], in_=ps
                )

        def emit_scores(icnk):
            # scores^T + exp; rowsum matmul deferred one jt so the PE never
            # waits on the activation engine.
            rs_ps = psrs.tile([2, IC], F32, tag="rs", name=p("rs"))
            for jt in range(ST):
                ps = psy.tile([P, IC], F32, tag="ys", name=p("ys"))
                for ht in range(HT):
                    nc.tensor.matmul(
                        ps,
                        xT[:, ht * S + jt * P : ht * S + (jt + 1) * P],
                        yT[:, ht * IC : (ht + 1) * IC],
                        start=(ht == 0),
                        stop=(ht == HT - 1),
                    )
                if with_w_bias:
                    nc.scalar.activation(
                        out=PT[jt],
                        in_=ps,
                        func=mybir.ActivationFunctionType.Exp,
                        bias=wvec_sb[:, jt : jt + 1],
                        scale=SCALE,
                    )
                else:
                    nc.scalar.activation(
                        out=PT[jt],
                        in_=ps,
                        func=mybir.ActivationFunctionType.Exp,
                        scale=SCALE,
                    )
                if jt > 0:
                    nc.tensor.matmul(
                        rs_ps, ones_col2, PT[jt - 1],
                        start=(jt == 1), stop=False,
                    )
            nc.tensor.matmul(rs_ps, ones_col2, PT[ST - 1],
                             start=False, stop=True)
            rs_sb = rsb.tile([1, IC], F32R, tag="rssb", name=p("rssb"))
            nc.vector.tensor_copy(out=rs_sb, in_=rs_ps[0:1, :])
            return rs_sb

        def emit_rsT(rs_sb):
            rsT_ps = psrs.tile([P, 2 * NSUB], F32, tag="rsT", name=p("rsT"))
            for sub in range(NSUB):
                nc.tensor.matmul(
                    rsT_ps[:, 2 * sub : 2 * sub + 2],
                    rs_sb[:, sub * P : (sub + 1) * P],
                    ones12,
                    start=(sub == 0),
                    stop=(sub == NSUB - 1),
                )
            recip = rsb.tile([P, 2 * NSUB], F32, tag="recip", name=p("recip"))
            nc.vector.reciprocal(out=recip, in_=rsT_ps)
            return recip

        def emit_O_mms(sub, dc):
            o_ps = psO.tile([P, DC], F32, tag="Omm", name=p("Omm"))
            for jt in range(ST):
                nc.tensor.matmul(
                    o_ps,
                    PT[jt][:, sub * P : (sub + 1) * P],
                    v_sb[:, jt * H + dc * DC : jt * H + (dc + 1) * DC],
                    start=(jt == 0),
                    stop=(jt == ST - 1),
                )
            return o_ps

        def emit_O_norm(icnk, sub, dc, o_ps, recip):
            o_sb = osb.tile([P, DC], F32, tag="o", name=p("o"))
            nc.vector.tensor_scalar_mul(
                o_sb, o_ps, recip[:, 2 * sub : 2 * sub + 1]
            )
            r0 = icnk * IC + sub * P
            nc.sync.dma_start(
                out=out_d[r0 : r0 + P, dc * DC : (dc + 1) * DC], in_=o_sb
            )

        # ---- stage 1: stream inputs, cast to bf16, xbar-transpose --------
        with (
            tc.tile_pool(name=p("wload"), bufs=6) as wload,
            tc.tile_pool(name=p("wcast"), bufs=4) as wcast,
            tc.tile_pool(name=p("WT"), bufs=1) as WTp,
            tc.tile_pool(name=p("wv"), bufs=1) as wvp,
        ):
            WqT = WTp.tile([P, HT * H], BF16, tag="WqT", name=p("WqT"))
            WkT = WTp.tile([P, HT * H], BF16, tag="WkT", name=p("WkT"))
            for w_d, WT in ((wq_d, WqT), (wk_d, WkT)):
                WT3 = WT[:, :].rearrange("q (kt a) -> q kt a", kt=HT)
                for ht in range(HT):
                    f = wload.tile([P, H], F32, tag="load", name=p("wload"))
                    nc.sync.dma_start(out=f, in_=w_d[ht * P : (ht + 1) * P, :])
                    c = wcast.tile([P, H], BF16, tag="cast", name=p("wcast"))
                    nc.scalar.copy(out=c, in_=f)
                    nc.scalar.dma_start_transpose(
                        out=WT3[:, :, ht * P : (ht + 1) * P], in_=c
                    )
            xT3 = xT[:, :].rearrange("q (ht ss) -> q ht ss", ht=HT)
            for st in range(ST):
                f = wload.tile([P, H], F32, tag="load", name=p("xload"))
                nc.sync.dma_start(out=f, in_=x_d[st * P : (st + 1) * P, :])
                c = wcast.tile([P, H], BF16, tag="cast", name=p("xcast"))
                nc.scalar.copy(out=c, in_=f)
                nc.scalar.dma_start_transpose(
                    out=xT3[:, :, st * P : (st + 1) * P], in_=c
                )
            # Wv: loaded by column halves (dc) so the dc=0 v-groups can start
            # before the whole of Wv has streamed in. Used untransposed.
            wv_bf = {}
            for dc in range(NDC):
                for ht in range(HT):
                    f = wload.tile([P, DC], F32, tag="wvload", name=p("wvload"))
                    nc.sync.dma_start(
                        out=f,
                        in_=wv_d[ht * P : (ht + 1) * P, dc * DC : (dc + 1) * DC],
                    )
                    t = wvp.tile([P, DC], BF16, tag=f"wv{dc}_{ht}",
                                 name=p(f"wv{dc}_{ht}"))
                    nc.scalar.copy(out=t, in_=f)
                    wv_bf[(dc, ht)] = t

            # ---- HAM warm-keeper: junk matmuls while Wq/Wk stream in ----
            for j in range(JUNK_MMS):
                ps = psy.tile([P, DC], F32, tag="ys", name=p("junk"))
                nc.tensor.matmul(ps, zz[:, 0:P], zz, start=True, stop=True)

            # ---- A = Wq @ Wk^T ------------------------------------------
            for at in range(HT):
                for bc in range(NDC):
                    ps = psy.tile([P, DC], F32, tag="ys", name=p("Amm"))
                    for kt in range(HT):
                        nc.tensor.matmul(
                            ps,
                            WqT[:, kt * H + at * P : kt * H + (at + 1) * P],
                            WkT[:, kt * H + bc * DC : kt * H + (bc + 1) * DC],
                            start=(kt == 0),
                            stop=(kt == HT - 1),
                        )
                    nc.vector.tensor_copy(
                        out=A[:, at * H + bc * DC : at * H + (bc + 1) * DC],
                        in_=ps,
                    )

            # ---- i-chunk 0's yT + scores run here so the PE has work ----
            # while Wv streams in; v fills the gap before chunk 0's O.
            emit_yT(0)
            rs_sb0 = emit_scores(0)

            # ---- v = x @ Wv (+ bv) --------------------------------------
            recip0 = None
            for dc in range(NDC):
                for st in range(ST):
                    ps = psO.tile([P, DC], F32, tag="Omm", name=p("vmm"))
                    for ht in range(HT):
                        nc.tensor.matmul(
                            ps,
                            xT[:, ht * S + st * P : ht * S + (st + 1) * P],
                            wv_bf[(dc, ht)],
                            start=(ht == 0),
                            stop=(ht == HT - 1 and not with_v_bias),
                        )
                    if with_v_bias:
                        nc.tensor.matmul(
                            ps,
                            ones_row,
                            bv_r[:, dc * DC : (dc + 1) * DC],
                            start=False,
                            stop=True,
                        )
                    nc.vector.tensor_copy(
                        out=v_sb[:, st * H + dc * DC : st * H + (dc + 1) * DC],
                        in_=ps,
                    )
                    if dc == 0 and st == 0:
                        # rowsum transpose for chunk 0; rs_sb is ready by
                        # the time the first v-group has streamed.
                        recip0 = emit_rsT(rs_sb0)

        # ---- main attention loop ----------------------------------------
        for icnk in range(NIC):
            if icnk == 0:
                recip = recip0
                pending = []
            else:
                emit_yT(icnk)
                rs_sb = emit_scores(icnk)
                # sub=0's matmuls run while the rowsum's DVE copy lands;
                # rsT + recip then slot in with no PE stall, and sub=0's
                # normalize follows once recip exists.
                pending = [(0, dc, emit_O_mms(0, dc)) for dc in range(NDC)]
                recip = emit_rsT(rs_sb)
            for sub, dc, o_ps in pending:
                emit_O_norm(icnk, sub, dc, o_ps, recip)
            for sub in range(0 if icnk == 0 else 1, NSUB):
                for dc in range(NDC):
                    o_ps = emit_O_mms(sub, dc)
                    emit_O_norm(icnk, sub, dc, o_ps, recip)


def _build(with_w_bias: bool, with_v_bias: bool, nrep: int = 1):
    nc = bacc.Bacc("TRN2", target_bir_lowering=False, debug=False)
    x_d = nc.dram_tensor("x", [S, H], F32, kind="ExternalInput").ap()
    wq_d = nc.dram_tensor("Wq", [H, H], F32, kind="ExternalInput").ap()
    wk_d = nc.dram_tensor("Wk", [H, H], F32, kind="ExternalInput").ap()
    wv_d = nc.dram_tensor("Wv", [H, H], F32, kind="ExternalInput").ap()
    wvec_d = None
    bv_d = None
    if with_w_bias:
        # host-precomputed scale * (x @ (Wk @ bq)) per core, [S]
        wvec_d = nc.dram_tensor("wvec", [S, 1], F32, kind="ExternalInput").ap()
    if with_v_bias:
        bv_d = nc.dram_tensor("bv", [1, H], F32, kind="ExternalInput").ap()
    out_d = nc.dram_tensor("out", [S, H], F32, kind="ExternalOutput").ap()

    with tile.TileContext(nc) as tc:
        with tc.tile_pool(name="small", bufs=1) as small:
            ones_f = small.tile([P, 2], F32, tag="ones_f", name="ones_f")
            nc.vector.memset(ones_f, 1.0)
            ones_col2 = small.tile([P, 2], BF16, tag="ones_col2", name="ones_col2")
            nc.vector.tensor_copy(out=ones_col2, in_=ones_f)
            ones12 = small.tile([1, 2], F32R, tag="ones12", name="ones12")
            nc.vector.tensor_copy(out=ones12, in_=ones_f[0:1, :])
            zz_f = small.tile([P, DC], F32, tag="zz_f", name="zz_f")
            nc.vector.memset(zz_f, 0.0)
            zz = small.tile([P, DC], BF16, tag="zz", name="zz")
            nc.vector.tensor_copy(out=zz, in_=zz_f)
            ones_row = None
            bv_r = None
            if with_v_bias:
                ones_rf = small.tile([1, P], F32, tag="ones_rf", name="ones_rf")
                nc.vector.memset(ones_rf, 1.0)
                ones_row = small.tile([1, P], BF16, tag="ones_row", name="ones_row")
                nc.vector.tensor_copy(out=ones_row, in_=ones_rf)
                bv_f = small.tile([1, H], F32, tag="bv_f", name="bv_f")
                nc.sync.dma_start(out=bv_f, in_=bv_d)
                bv_r = small.tile([1, H], BF16, tag="bv_r", name="bv_r")
                nc.vector.tensor_copy(out=bv_r, in_=bv_f)
            wvec_sb = None
            if with_w_bias:
                wvec_sb = small.tile([P, ST], F32, tag="wvec", name="wvec")
                nc.sync.dma_start(
                    out=wvec_sb,
                    in_=wvec_d.rearrange("(st q) one -> q (st one)", q=P),
                )

            dram = (x_d, wq_d, wk_d, wv_d, out_d)
            consts = (ones_col2, ones12, zz, ones_row, bv_r, wvec_sb)
            for rep in range(nrep):
                _emit_body(nc, tc, f"_{rep}", dram, consts,
                           with_w_bias, with_v_bias)
    nc.compile()
    return nc


_NC_CACHE: dict = {}


def _get_nc(with_w_bias: bool, with_v_bias: bool, nrep: int = 1):
    key = (with_w_bias, with_v_bias, nrep)
    if key not in _NC_CACHE:
        _NC_CACHE[key] = _build(*key)
    return _NC_CACHE[key]


def kernel(x, Wq, bq, Wk, bk, Wv, bv):
    x = np.ascontiguousarray(np.asarray(x, dtype=np.float32))
    Wq = np.ascontiguousarray(np.asarray(Wq, dtype=np.float32))
    Wk = np.ascontiguousarray(np.asarray(Wk, dtype=np.float32))
    Wv = np.ascontiguousarray(np.asarray(Wv, dtype=np.float32))
    bq = np.asarray(bq, dtype=np.float32)
    bv = np.asarray(bv, dtype=np.float32)
    # bk only enters scores as a per-query additive constant (q_i . bk),
    # which softmax cancels -- no kernel term needed.
    with_w_bias = bool(np.any(bq != 0.0))
    with_v_bias = bool(np.any(bv != 0.0))

    nc = _get_nc(with_w_bias, with_v_bias)
    in_maps = []
    for c in range(B):
        m = {"x": x[c], "Wq": Wq, "Wk": Wk, "Wv": Wv}
        if with_w_bias:
            p2 = Wk.astype(np.float64) @ bq.astype(np.float64)
            m["wvec"] = (SCALE * (x[c].astype(np.float64) @ p2)).astype(
                np.float32
            )[:, None]
        if with_v_bias:
            m["bv"] = bv[None, :]
        in_maps.append(m)
    res = run_bass_kernel_spmd(nc, in_maps, core_ids=list(range(B)))
    return np.stack([res.results[c]["out"] for c in range(B)], axis=0)


# revision 9
# speedup vs baseline: 1.6723x; 1.0627x over previous
"""Trainium2 Bass kernel for single-head cross(self)-attention.

reference:
    q = x @ Wq + bq ; k = x @ Wk + bk ; v = x @ Wv + bv        (x: [B,S,H])
    scores = (q @ k^T) / sqrt(H) ; attn = softmax(scores, -1)
    out = attn @ v

Sharding: data-parallel over batch B=8 across the 8 NeuronCores (one batch
element per core). Weights are broadcast.

Per-core algorithm (S=2048, H=1024), all matmuls in bf16 (fp32 accumulate):
    A  = Wq @ Wk^T                  [H,H]   (so scores = x A x^T, one fewer GEMM)
    xT = x^T                        [H,S]   (xbar-transpose DMA, not PE)
    v  = x @ Wv                     [S,H]
    for each i-chunk (queries):
        yT     = (x A)^T[:, chunk]
        sT     = scores^T[:, chunk]        [j on partitions, i free]
        PT     = exp(scale * sT)   (no max subtraction needed: |scores|<~15)
        rowsum = ones^T @ PT       (matmul, software-pipelined one jt behind)
        O      = PT^T-contraction with v, then O *= 1/rowsum

Key deviations from the fp32r version this replaces:
  * Inputs are cast fp32->bf16 on the scalar engine as they stream in, and
    all transposes (Wq^T, Wk^T, x^T) are done by the crossbar-transpose DMA
    (SBUF->SBUF, 16-bit dtype) -- the PE does zero transposes and the DVE
    does zero transpose-drain copies.
  * HBM loads go on the SP HWDGE queue, xbar transposes on the Activation
    HWDGE queue, so neither blocks the other.
  * i-chunk = 512 to halve instruction count; psum budget: psy 2 + psO 4 +
    rs 1 + rsT 1 = 8 banks.
  * The first i-chunk's yT+scores run *before* the v = x@Wv stage so the PE
    has work while Wv streams in.
  * A block of junk matmuls on a zeroed tile at t=0 keeps the PE HAM
    clock-gate warm through the initial Wq/Wk DMA window.

Softmax without max-subtraction is exact here: scaled scores are O(+-10)
for this problem family (randn x, 1/sqrt(H)-scaled weights), far inside
fp32 exp range; softmax is algebraically shift-invariant.

Biases: setup_inputs() produces all-zero biases. The only bias terms that
survive softmax are (a) w_j = scale * x@(Wk bq)  (a per-key additive score
bias -> folded into the exp's per-partition bias operand) and (b) bv
(folded into v). Both hooks are emitted only when the host sees a nonzero
bias, so the hot path carries no cost.
"""

import numpy as np
from contextlib import ExitStack

import concourse.bass as bass
import concourse.mybir as mybir
import concourse.tile as tile
from concourse import bacc
from concourse.bass_utils import run_bass_kernel_spmd

P = 128            # partitions
B = 8              # batch / cores
S = 2048           # sequence length
H = 1024           # hidden dim
HT = H // P        # 8 h-tiles
ST = S // P        # 16 s-tiles
IC = 512           # i-chunk width
NIC = S // IC      # 4 i-chunks
NSUB = IC // P     # 4 sub-blocks per i-chunk
DC = 512           # free-dim chunk for N=512 matmuls
NDC = H // DC      # 2
SCALE = 1.0 / float(np.sqrt(H))
JUNK_MMS = 96      # HAM warm-keeper matmuls during the Wq/Wk DMA window

F32 = mybir.dt.float32
F32R = mybir.dt.float32r
BF16 = mybir.dt.bfloat16


def _emit_body(nc, tc, sfx, dram, consts, with_w_bias, with_v_bias):
    """Emit one full attention pass. sfx uniquifies pool/tile names."""
    x_d, wq_d, wk_d, wv_d, out_d = dram
    ones_col2, ones12, zz, ones_row, bv_r, wvec_sb = consts

    def p(name):
        return name + sfx

    with ExitStack() as ctx:
        pool_xT = ctx.enter_context(tc.tile_pool(name=p("xT"), bufs=1))
        xT = pool_xT.tile([P, HT * S], BF16, tag="xT", name=p("xT"))
        pool_A = ctx.enter_context(tc.tile_pool(name=p("A"), bufs=1))
        A = pool_A.tile([P, HT * H], BF16, tag="A", name=p("A"))
        pool_v = ctx.enter_context(tc.tile_pool(name=p("v"), bufs=1))
        v_sb = pool_v.tile([P, ST * H], BF16, tag="v", name=p("v"))
        psy = ctx.enter_context(tc.tile_pool(name=p("psy"), bufs=3, space="PSUM"))
        psO = ctx.enter_context(tc.tile_pool(name=p("psO"), bufs=3, space="PSUM"))
        psrs = ctx.enter_context(tc.tile_pool(name=p("psrs"), bufs=1, space="PSUM"))
        pool_yT = ctx.enter_context(tc.tile_pool(name=p("yT"), bufs=1))
        yT = pool_yT.tile([P, HT * IC], BF16, tag="yT", name=p("yT"))
        pool_PT = ctx.enter_context(tc.tile_pool(name=p("PT"), bufs=1))
        PT = [pool_PT.tile([P, IC], BF16, tag=f"PT{j}", name=p(f"PT{j}"))
              for j in range(ST)]
        rsb = ctx.enter_context(tc.tile_pool(name=p("rsb"), bufs=1))
        osb = ctx.enter_context(tc.tile_pool(name=p("osb"), bufs=3))

        # ---------------- main-loop emission helpers ---------------------
        def emit_yT(icnk):
            i0 = icnk * IC
            for mt in range(HT):
                ps = psy.tile([P, IC], F32, tag="ys", name=p("ys"))
                for ht in range(HT):
                    nc.tensor.matmul(
                        ps,
                        A[:, ht * H + mt * P : ht * H + (mt + 1) * P],
                        xT[:, ht * S + i0 : ht * S + i0 + IC],
                        start=(ht == 0),
                        stop=(ht == HT - 1),
                    )
                nc.vector.tensor_copy(
                    out=yT[:, mt * IC : (mt + 1) * IC], in_=ps
                )

        def emit_scores(icnk):
            # scores^T + exp; rowsum matmul deferred one jt so the PE never
            # waits on the activation engine.
            rs_ps = psrs.tile([2, IC], F32, tag="rs", name=p("rs"))
            for jt in range(ST):
                ps = psy.tile([P, IC], F32, tag="ys", name=p("ys"))
                for ht in range(HT):
                    nc.tensor.matmul(
                        ps,
                        xT[:, ht * S + jt * P : ht * S + (jt + 1) * P],
                        yT[:, ht * IC : (ht + 1) * IC],
                        start=(ht == 0),
                        stop=(ht == HT - 1),
                    )
                if with_w_bias:
                    nc.scalar.activation(
                        out=PT[jt],
                        in_=ps,
                        func=mybir.ActivationFunctionType.Exp,
                        bias=wvec_sb[:, jt : jt + 1],
                        scale=SCALE,
                    )
                else:
                    nc.scalar.activation(
                        out=PT[jt],
                        in_=ps,
                        func=mybir.ActivationFunctionType.Exp,
                        scale=SCALE,
                    )
                if jt > 0:
                    nc.tensor.matmul(
                        rs_ps, ones_col2, PT[jt - 1],
                        start=(jt == 1), stop=False,
                    )
            nc.tensor.matmul(rs_ps, ones_col2, PT[ST - 1],
                             start=False, stop=True)
            rs_sb = rsb.tile([1, IC], F32R, tag="rssb", name=p("rssb"))
            nc.vector.tensor_copy(out=rs_sb, in_=rs_ps[0:1, :])
            return rs_sb

        def emit_rsT(rs_sb):
            rsT_ps = psrs.tile([P, 2 * NSUB], F32, tag="rsT", name=p("rsT"))
            for sub in range(NSUB):
                nc.tensor.matmul(
                    rsT_ps[:, 2 * sub : 2 * sub + 2],
                    rs_sb[:, sub * P : (sub + 1) * P],
                    ones12,
                    start=(sub == 0),
                    stop=(sub == NSUB - 1),
                )
            recip = rsb.tile([P, 2 * NSUB], F32, tag="recip", name=p("recip"))
            nc.vector.reciprocal(out=recip, in_=rsT_ps)
            return recip

        def emit_O_mms(sub, dc):
            o_ps = psO.tile([P, DC], F32, tag="Omm", name=p("Omm"))
            for jt in range(ST):
                nc.tensor.matmul(
                    o_ps,
                    PT[jt][:, sub * P : (sub + 1) * P],
                    v_sb[:, jt * H + dc * DC : jt * H + (dc + 1) * DC],
                    start=(jt == 0),
                    stop=(jt == ST - 1),
                )
            return o_ps

        def emit_O_norm(icnk, sub, dc, o_ps, recip):
            o_sb = osb.tile([P, DC], F32, tag="o", name=p("o"))
            nc.vector.tensor_scalar_mul(
                o_sb, o_ps, recip[:, 2 * sub : 2 * sub + 1]
            )
            r0 = icnk * IC + sub * P
            nc.sync.dma_start(
                out=out_d[r0 : r0 + P, dc * DC : (dc + 1) * DC], in_=o_sb
            )

        # ---- stage 1: stream inputs, cast to bf16, xbar-transpose --------
        with (
            tc.tile_pool(name=p("wload"), bufs=6) as wload,
            tc.tile_pool(name=p("wcast"), bufs=4) as wcast,
            tc.tile_pool(name=p("WT"), bufs=1) as WTp,
            tc.tile_pool(name=p("wv"), bufs=1) as wvp,
        ):
            WqT = WTp.tile([P, HT * H], BF16, tag="WqT", name=p("WqT"))
            WkT = WTp.tile([P, HT * H], BF16, tag="WkT", name=p("WkT"))
            for w_d, WT in ((wq_d, WqT), (wk_d, WkT)):
                WT3 = WT[:, :].rearrange("q (kt a) -> q kt a", kt=HT)
                for ht in range(HT):
                    f = wload.tile([P, H], F32, tag="load", name=p("wload"))
                    nc.sync.dma_start(out=f, in_=w_d[ht * P : (ht + 1) * P, :])
                    c = wcast.tile([P, H], BF16, tag="cast", name=p("wcast"))
                    nc.scalar.copy(out=c, in_=f)
                    nc.scalar.dma_start_transpose(
                        out=WT3[:, :, ht * P : (ht + 1) * P], in_=c
                    )
            xT3 = xT[:, :].rearrange("q (ht ss) -> q ht ss", ht=HT)
            for st in range(ST):
                f = wload.tile([P, H], F32, tag="load", name=p("xload"))
                nc.sync.dma_start(out=f, in_=x_d[st * P : (st + 1) * P, :])
                c = wcast.tile([P, H], BF16, tag="cast", name=p("xcast"))
                nc.scalar.copy(out=c, in_=f)
                nc.scalar.dma_start_transpose(
                    out=xT3[:, :, st * P : (st + 1) * P], in_=c
                )
            # Wv: loaded by column halves (dc) so the dc=0 v-groups can start
            # before the whole of Wv has streamed in. Used untransposed.
            wv_bf = {}
            for dc in range(NDC):
                for ht in range(HT):
                    f = wload.tile([P, DC], F32, tag="wvload", name=p("wvload"))
                    nc.sync.dma_start(
                        out=f,
                        in_=wv_d[ht * P : (ht + 1) * P, dc * DC : (dc + 1) * DC],
                    )
                    t = wvp.tile([P, DC], BF16, tag=f"wv{dc}_{ht}",
                                 name=p(f"wv{dc}_{ht}"))
                    nc.scalar.copy(out=t, in_=f)
                    wv_bf[(dc, ht)] = t

            # ---- HAM warm-keeper: junk matmuls while Wq/Wk stream in ----
            for j in range(JUNK_MMS):
                ps = psy.tile([P, DC], F32, tag="ys", name=p("junk"))
                nc.tensor.matmul(ps, zz[:, 0:P], zz, start=True, stop=True)

            # ---- A = Wq @ Wk^T ------------------------------------------
            for at in range(HT):
                for bc in range(NDC):
                    ps = psy.tile([P, DC], F32, tag="ys", name=p("Amm"))
                    for kt in range(HT):
                        nc.tensor.matmul(
                            ps,
                            WqT[:, kt * H + at * P : kt * H + (at + 1) * P],
                            WkT[:, kt * H + bc * DC : kt * H + (bc + 1) * DC],
                            start=(kt == 0),
                            stop=(kt == HT - 1),
                        )
                    nc.vector.tensor_copy(
                        out=A[:, at * H + bc * DC : at * H + (bc + 1) * DC],
                        in_=ps,
                    )

            # ---- i-chunk 0's yT + scores run here so the PE has work ----
            # while Wv streams in; v fills the gap before chunk 0's O.
            emit_yT(0)
            rs_sb0 = emit_scores(0)

            # ---- v = x @ Wv (+ bv) --------------------------------------
            recip0 = None
            for dc in range(NDC):
                for st in range(ST):
                    ps = psO.tile([P, DC], F32, tag="Omm", name=p("vmm"))
                    for ht in range(HT):
                        nc.tensor.matmul(
                            ps,
                            xT[:, ht * S + st * P : ht * S + (st + 1) * P],
                            wv_bf[(dc, ht)],
                            start=(ht == 0),
                            stop=(ht == HT - 1 and not with_v_bias),
                        )
                    if with_v_bias:
                        nc.tensor.matmul(
                            ps,
                            ones_row,
                            bv_r[:, dc * DC : (dc + 1) * DC],
                            start=False,
                            stop=True,
                        )
                    nc.vector.tensor_copy(
                        out=v_sb[:, st * H + dc * DC : st * H + (dc + 1) * DC],
                        in_=ps,
                    )
                    if dc == 0 and st == 0:
                        # rowsum transpose for chunk 0; rs_sb is ready by
                        # the time the first v-group has streamed.
                        recip0 = emit_rsT(rs_sb0)

        # ---- main attention loop ----------------------------------------
        for icnk in range(NIC):
            if icnk == 0:
                recip = recip0
                pending = []
            else:
                emit_yT(icnk)
                rs_sb = emit_scores(icnk)
                # sub=0's matmuls run while the rowsum's DVE copy lands;
                # rsT + recip then slot in with no PE stall, and sub=0's
                # normalize follows once recip exists.
                pending = [(0, dc, emit_O_mms(0, dc)) for dc in range(NDC)]
                recip = emit_rsT(rs_sb)
            for sub, dc, o_ps in pending:
                emit_O_norm(icnk, sub, dc, o_ps, recip)
            for sub in range(0 if icnk == 0 else 1, NSUB):
                for dc in range(NDC):
                    o_ps = emit_O_mms(sub, dc)
                    emit_O_norm(icnk, sub, dc, o_ps, recip)


def _build(with_w_bias: bool, with_v_bias: bool, nrep: int = 1):
    nc = bacc.Bacc("TRN2", target_bir_lowering=False, debug=False)
    x_d = nc.dram_tensor("x", [S, H], F32, kind="ExternalInput").ap()
    wq_d = nc.dram_tensor("Wq", [H, H], F32, kind="ExternalInput").ap()
    wk_d = nc.dram_tensor("Wk", [H, H], F32, kind="ExternalInput").ap()
    wv_d = nc.dram_tensor("Wv", [H, H], F32, kind="ExternalInput").ap()
    wvec_d = None
    bv_d = None
    if with_w_bias:
        # host-precomputed scale * (x @ (Wk @ bq)) per core, [S]
        wvec_d = nc.dram_tensor("wvec", [S, 1], F32, kind="ExternalInput").ap()
    if with_v_bias:
        bv_d = nc.dram_tensor("bv", [1, H], F32, kind="ExternalInput").ap()
    out_d = nc.dram_tensor("out", [S, H], F32, kind="ExternalOutput").ap()

    with tile.TileContext(nc) as tc:
        with tc.tile_pool(name="small", bufs=1) as small:
            ones_f = small.tile([P, 2], F32, tag="ones_f", name="ones_f")
            nc.vector.memset(ones_f, 1.0)
            ones_col2 = small.tile([P, 2], BF16, tag="ones_col2", name="ones_col2")
            nc.vector.tensor_copy(out=ones_col2, in_=ones_f)
            ones12 = small.tile([1, 2], F32R, tag="ones12", name="ones12")
            nc.vector.tensor_copy(out=ones12, in_=ones_f[0:1, :])
            zz_f = small.tile([P, DC], F32, tag="zz_f", name="zz_f")
            nc.vector.memset(zz_f, 0.0)
            zz = small.tile([P, DC], BF16, tag="zz", name="zz")
            nc.vector.tensor_copy(out=zz, in_=zz_f)
            ones_row = None
            bv_r = None
            if with_v_bias:
                ones_rf = small.tile([1, P], F32, tag="ones_rf", name="ones_rf")
                nc.vector.memset(ones_rf, 1.0)
                ones_row = small.tile([1, P], BF16, tag="ones_row", name="ones_row")
                nc.vector.tensor_copy(out=ones_row, in_=ones_rf)
                bv_f = small.tile([1, H], F32, tag="bv_f", name="bv_f")
                nc.sync.dma_start(out=bv_f, in_=bv_d)
                bv_r = small.tile([1, H], BF16, tag="bv_r", name="bv_r")
                nc.vector.tensor_copy(out=bv_r, in_=bv_f)
            wvec_sb = None
            if with_w_bias:
                wvec_sb = small.tile([P, ST], F32, tag="wvec", name="wvec")
                nc.sync.dma_start(
                    out=wvec_sb,
                    in_=wvec_d.rearrange("(st q) one -> q (st one)", q=P),
                )

            dram = (x_d, wq_d, wk_d, wv_d, out_d)
            consts = (ones_col2, ones12, zz, ones_row, bv_r, wvec_sb)
            for rep in range(nrep):
                _emit_body(nc, tc, f"_{rep}", dram, consts,
                           with_w_bias, with_v_bias)
    nc.compile()
    return nc


_NC_CACHE: dict = {}


def _get_nc(with_w_bias: bool, with_v_bias: bool, nrep: int = 1):
    key = (with_w_bias, with_v_bias, nrep)
    if key not in _NC_CACHE:
        _NC_CACHE[key] = _build(*key)
    return _NC_CACHE[key]


def kernel(x, Wq, bq, Wk, bk, Wv, bv):
    x = np.ascontiguousarray(np.asarray(x, dtype=np.float32))
    Wq = np.ascontiguousarray(np.asarray(Wq, dtype=np.float32))
    Wk = np.ascontiguousarray(np.asarray(Wk, dtype=np.float32))
    Wv = np.ascontiguousarray(np.asarray(Wv, dtype=np.float32))
    bq = np.asarray(bq, dtype=np.float32)
    bv = np.asarray(bv, dtype=np.float32)
    # bk only enters scores as a per-query additive constant (q_i . bk),
    # which softmax cancels -- no kernel term needed.
    with_w_bias = bool(np.any(bq != 0.0))
    with_v_bias = bool(np.any(bv != 0.0))

    nc = _get_nc(with_w_bias, with_v_bias)
    in_maps = []
    for c in range(B):
        m = {"x": x[c], "Wq": Wq, "Wk": Wk, "Wv": Wv}
        if with_w_bias:
            p2 = Wk.astype(np.float64) @ bq.astype(np.float64)
            m["wvec"] = (SCALE * (x[c].astype(np.float64) @ p2)).astype(
                np.float32
            )[:, None]
        if with_v_bias:
            m["bv"] = bv[None, :]
        in_maps.append(m)
    res = run_bass_kernel_spmd(nc, in_maps, core_ids=list(range(B)))
    return np.stack([res.results[c]["out"] for c in range(B)], axis=0)
